# revision 1
# baseline (speedup 1.0000x reference)
"""Trainium2 Bass kernel for nn_BiLSTM_57440892617018.

2-layer bidirectional LSTM (independent fw / bw stacks, merge_mode='ave'),
B=2048, T=200, D=U=128. Data-parallel over batch across 8 NeuronCores.

Per-core structure: ONE software-pipelined loop with 4 recurrent streams,
parity-staggered: even supersteps advance the forward pair (L0f, L1f — layer
1 lagging layer 0 by one timestep), odd supersteps advance the backward pair
on the time-reversed sequence. The stagger gives every stream's recurrence
chain two superstep periods of latency budget, so pair-merged ACT
instructions (sigmoid over [i0|i1|f0|f1|o0|o1], tanh over [g0|g1], tanh over
[c0|c1]) stay off the tight chain. The tail of each superstep (tanh(c),
h-mul, merge, store) is emitted one block later so the ACT queue interleaves
the two parities with no idle window. All matmuls fp16 on PE (fp32 PSUM
accumulation), elementwise fp16 on DVE at 2x mode. Layer-1 outputs are
staged in SBUF (half of each direction) and merged on PE via 0.5*I matmuls,
which also yields the [b, d] layout the output DMA needs.
"""

import numpy as np
import ml_dtypes

import concourse.bass as bass
import concourse.tile as tile
from concourse import bacc, mybir
from concourse.bass_utils import run_bass_kernel_spmd

F32 = mybir.dt.float32
DT2 = mybir.dt.float16
DT2_NP = np.float16
AF = mybir.ActivationFunctionType

B, T, D, U = 2048, 200, 128, 128
NCORES = 8
BL = B // NCORES          # 256 batch per core
NB = BL // 128            # 2 b-tiles
BCOL = 128 * NB           # 256 free columns (batch)

# gate order inside the reference 4u axis: i, f, g, o
GATE_COLS = {"i": 0, "f": 128, "g": 256, "o": 384}
SLOTS = ["Wi", "Wf", "Wo", "Wg", "Ui", "Uf", "Uo", "Ug"]

_cache = {}


def _wcol(di, l, slot):
    return ((di * 2 + l) * 8 + slot) * 128


def _build(uniform_bias, bias_val, Tn=T, reps=1, loop_R=0, tiny_x=False,
           split_o=True):
    nc = bacc.Bacc("TRN2", target_bir_lowering=False, debug=False,
                   num_devices=NCORES)
    Th = Tn // 2

    xs = nc.dram_tensor("xs", [BL, 1 if tiny_x else Tn, D], F32,
                        kind="ExternalInput")
    wts = nc.dram_tensor("wts", [2, 2, 8, 128, 128], DT2, kind="ExternalInput")
    ident2 = nc.dram_tensor("ident2", [128, 256], DT2, kind="ExternalInput")
    biases = nc.dram_tensor("biases", [128, 16], F32, kind="ExternalInput")
    if loop_R:
        dummyout = nc.dram_tensor("dummyout", [128, 4], F32,
                                  kind="ExternalOutput")
    else:
        out = nc.dram_tensor("out", [BL, Tn, D], F32, kind="ExternalOutput")

    NS = 2 * Tn + 2   # supersteps

    with tile.TileContext(nc) as tc:
        with (
            tc.tile_pool(name="wpool", bufs=1) as wpool,
            tc.tile_pool(name="stage", bufs=1) as stage,
            tc.tile_pool(name="xraw", bufs=6) as xraw,
            tc.tile_pool(name="xtp", bufs=4) as xtp,
            tc.tile_pool(name="hpool", bufs=6) as hpool,
            tc.tile_pool(name="cpool", bufs=1) as cpool,
            tc.tile_pool(name="gsb", bufs=3) as gsb,
            tc.tile_pool(name="tcp", bufs=4) as tcp,
            tc.tile_pool(name="tmp", bufs=8) as tmp,
            tc.tile_pool(name="outp", bufs=3) as outp,
            tc.tile_pool(name="dramp", bufs=1, space="DRAM") as dramp,
            tc.tile_pool(name="psig", bufs=2, space="PSUM") as psig,
            tc.tile_pool(name="pg", bufs=1, space="PSUM") as pg,
            tc.tile_pool(name="pscr", bufs=1, space="PSUM") as pscr,
        ):
            # ---- constants / weights ----
            wslab = wpool.tile([128, 4096], DT2)
            for di in range(2):
                for l in range(2):
                    base = _wcol(di, l, 0)
                    nc.sync.dma_start(
                        wslab[:, base:base + 1024].rearrange(
                            "k (s m) -> k s m", s=8),
                        wts.ap()[di, l].rearrange("s k m -> k s m"))
            idt = wpool.tile([128, 256], DT2)
            nc.sync.dma_start(idt[:], ident2.ap())
            eye = idt[:, 0:128]
            half_eye = idt[:, 128:256]
            bsb = wpool.tile([128, 16], F32)
            nc.sync.dma_start(bsb[:], biases.ap())

            if loop_R:
                out_int = dramp.tile([BL, Tn, D], F32)

                def out_ap():
                    return out_int[:]
            else:
                def out_ap():
                    return out.ap()

            z0 = wpool.tile([128, BCOL], DT2)
            nc.gpsimd.memset(z0[:], 0.0)

            # persistent cell states per direction: [c0|c1] pair tiles
            c_pair = {}
            for di in range(2):
                ct = cpool.tile([128, 2 * BCOL], DT2, tag=f"c{di}")
                nc.gpsimd.memset(ct[:], 0.0)
                c_pair[di] = ct

            # SBUF staging for layer-1 outputs awaiting merge
            stage_f = stage.tile([128, Th * BCOL], DT2, tag="stf")
            stage_b = stage.tile([128, Th * BCOL], DT2, tag="stb")

            # scratch PSUM bank: f32 cols [0:128)+[128:256) = 2 parity slots
            # for x transposes (fp16 via bitcast); [256:512) = merge psum
            scr = pscr.tile([128, 512], F32)

            h_prev = {(0, 0): z0[:], (0, 1): z0[:],
                      (1, 0): z0[:], (1, 1): z0[:]}
            xT_ready = {}
            xr_ready = {}

            def x_load(s):
                m = s // 2
                if m >= Tn:
                    return
                di = s % 2
                tsrc = 0 if tiny_x else (m if di == 0 else Tn - 1 - m)
                with nc.named_scope("xload"):
                    xr = xraw.tile([128, NB, 128], F32)
                    nc.sync.dma_start(
                        xr[:],
                        xs.ap()[:, tsrc, :].rearrange("(j p) d -> p j d",
                                                      j=NB))
                xr_ready[s] = xr

            def x_prep(s):
                m = s // 2
                if m >= Tn:
                    return
                xr = xr_ready.pop(s)
                sc = nc.named_scope("xprep"); sc.__enter__()
                xb = xtp.tile([128, BCOL], DT2, tag="xb")
                nc.vector.tensor_copy(
                    xb[:], xr[:].rearrange("p j d -> p (j d)"))
                par = (s % 2) * 128
                xpd = scr[:, par:par + 128].bitcast(DT2)   # [128, 256] fp16
                nc.tensor.transpose(xpd[:, 0:128], xb[:, 0:128], eye)
                nc.tensor.transpose(xpd[:, 128:256], xb[:, 128:256], eye)
                xT = xtp.tile([128, BCOL], DT2, tag="xT")
                nc.vector.tensor_copy(xT[:], xpd[:])
                sc.__exit__(None, None, None)
                xT_ready[s] = xT

            def finish_head(pend):
                """Early tail of superstep s-1: tanh(c) and the h products."""
                (di, st0, st1, tt1, gates, lo, hi) = pend
                c_t = c_pair[di]
                tc_t = tcp.tile([128, 2 * BCOL], DT2)
                with nc.named_scope("tanhc"):
                    nc.scalar.activation(tc_t[:, lo:hi], c_t[:, lo:hi],
                                         AF.Tanh)
                merge = None
                if st0:
                    h_t = hpool.tile([128, BCOL], DT2, tag="h0")
                    with nc.named_scope("hmul"):
                        nc.vector.tensor_mul(h_t[:], gates[:, 1024:1280],
                                             tc_t[:, 0:BCOL])
                    h_prev[(di, 0)] = h_t[:]
                if st1:
                    o_ap = gates[:, 1280:1536]
                    stg = stage_f if di == 0 else stage_b
                    stage_this = (tt1 < Th) if di == 0 else (tt1 >= Th)
                    if stage_this:
                        soff = (tt1 if di == 0 else tt1 - Th) * BCOL
                        dst = stg[:, soff:soff + BCOL]
                        nc.vector.tensor_mul(dst, o_ap, tc_t[:, BCOL:2 * BCOL])
                        h_prev[(di, 1)] = dst
                    else:
                        h_t = hpool.tile([128, BCOL], DT2, tag="h1")
                        nc.vector.tensor_mul(h_t[:], o_ap,
                                             tc_t[:, BCOL:2 * BCOL])
                        h_prev[(di, 1)] = h_t[:]
                        merge = (di, tt1, h_t)
                return merge

            def finish_tail(merge):
                """Late tail of superstep s-1: output merge + store."""
                di, tt1, h_t = merge
                sc = nc.named_scope("mergeout"); sc.__enter__()
                ostg = stage_b if di == 0 else stage_f
                ooff = (tt1 - Th if di == 0 else tt1) * BCOL
                other = ostg[:, ooff:ooff + BCOL]
                for j in range(NB):
                    mdst = scr[:, 256 + j * 128:256 + j * 128 + 128]
                    nc.tensor.matmul(
                        mdst, h_t[:, j * 128:j * 128 + 128],
                        half_eye, start=True, stop=False)
                    nc.tensor.matmul(
                        mdst, other[:, j * 128:j * 128 + 128],
                        half_eye, start=False, stop=True)
                ost = outp.tile([128, BCOL], F32)
                nc.vector.tensor_copy(ost[:], scr[:, 256:512])
                nc.sync.dma_start(
                    out_ap()[:, tt1, :].rearrange("(j p) d -> p j d", j=NB),
                    ost[:].rearrange("p (j d) -> p j d", j=NB))
                sc.__exit__(None, None, None)

            rep_loop = True
            pending = None          # ACT/DVE state of superstep s-1
            mm_ctx = {}             # s -> (sig_ps, g_ps) emitted matmuls

            def emit_mms(s):
                """Emit all gate matmuls for superstep s (sig-feeding first)."""
                di = s % 2
                m = s // 2
                st0 = m < Tn
                st1 = 1 <= m <= Tn
                if not (st0 or st1) or s >= NS:
                    return
                sc = nc.named_scope("gatemm"); sc.__enter__()
                sig_ps = psig.tile([128, 1536], F32)
                g_ps = pg.tile([128, 512], F32)
                rhs = {}
                for stream, active in ((0, st0), (1, st1)):
                    if active:
                        rhs[stream] = (
                            xT_ready.pop(s) [:] if stream == 0
                            else h_prev[(di, 0)],
                            h_prev[(di, stream)])
                for gi in (0, 1):     # i, f feed the early sigmoid
                    for stream in rhs:
                        dst = sig_ps[:, gi * 512 + stream * 256:
                                     gi * 512 + stream * 256 + 256]
                        nc.tensor.matmul(
                            dst, wslab[:, _wcol(di, stream, gi):
                                       _wcol(di, stream, gi) + 128],
                            rhs[stream][0], start=True, stop=False)
                        nc.tensor.matmul(
                            dst, wslab[:, _wcol(di, stream, 4 + gi):
                                       _wcol(di, stream, 4 + gi) + 128],
                            rhs[stream][1], start=False, stop=True)
                for stream in rhs:
                    dstg = g_ps[:, stream * 256:stream * 256 + 256]
                    nc.tensor.matmul(
                        dstg, wslab[:, _wcol(di, stream, 3):
                                    _wcol(di, stream, 3) + 128],
                        rhs[stream][0], start=True, stop=False)
                    nc.tensor.matmul(
                        dstg, wslab[:, _wcol(di, stream, 7):
                                    _wcol(di, stream, 7) + 128],
                        rhs[stream][1], start=False, stop=True)
                gi = 2                # o gates: off the critical chain
                for stream in rhs:
                    dst = sig_ps[:, gi * 512 + stream * 256:
                                 gi * 512 + stream * 256 + 256]
                    nc.tensor.matmul(
                        dst, wslab[:, _wcol(di, stream, gi):
                                   _wcol(di, stream, gi) + 128],
                        rhs[stream][0], start=True, stop=False)
                    nc.tensor.matmul(
                        dst, wslab[:, _wcol(di, stream, 4 + gi):
                                   _wcol(di, stream, 4 + gi) + 128],
                        rhs[stream][1], start=False, stop=True)
                sc.__exit__(None, None, None)
                mm_ctx[s] = (sig_ps, g_ps)

            import contextlib
            if loop_R:
                cnt = wpool.tile([128, 4], F32, tag="cnt")
                nc.gpsimd.memset(cnt[:], 0.0)
            loop_cm = tc.For_i(0, loop_R, 1) if loop_R else \
                contextlib.nullcontext()
            with loop_cm:
             if loop_R:
                 nc.vector.tensor_scalar_add(cnt[:], cnt[:], 1.0)
             for rep in range(reps):
              if rep > 0 or loop_R:
                for di_ in range(2):
                    nc.gpsimd.memset(c_pair[di_][:], 0.0)
                h_prev.update({(0, 0): z0[:], (0, 1): z0[:],
                               (1, 0): z0[:], (1, 1): z0[:]})
                pending = None
              for s in range(NS):
                di = s % 2
                m = s // 2
                st0 = m < Tn
                st1 = 1 <= m <= Tn
                t1 = m - 1
                tt1 = t1 if di == 0 else Tn - 1 - t1

                # ---- prologue / x prefetch ----
                if s == 0:
                    for ps in range(4):
                        x_load(ps)
                    x_prep(0)
                    x_prep(1)
                    emit_mms(0)
                if s + 4 < NS:
                    x_load(s + 4)

                # ---- early tail of superstep s-1 (other parity) ----
                merge = None
                if pending is not None:
                    merge = finish_head(pending)
                    pending = None

                # ---- activations for superstep s ----
                sig_ps, g_ps = mm_ctx.pop(s)
                gates = gsb.tile([128, 2048], DT2)
                lo = 0 if st0 else BCOL
                hi = 2 * BCOL if st1 else BCOL
                sig_o = None
                if uniform_bias and st0 and st1:
                    ncol = 1024 if split_o else 1536
                    with nc.named_scope("sig"):
                        nc.scalar.activation(gates[:, 0:ncol],
                                             sig_ps[:, 0:ncol],
                                             AF.Sigmoid, bias=bias_val)
                    with nc.named_scope("tanhg"):
                        nc.scalar.activation(gates[:, 1536:2048], g_ps[:],
                                             AF.Tanh, bias=bias_val)
                    if split_o:
                        sig_o = (gates, sig_ps)
                else:
                    for stream, active in ((0, st0), (1, st1)):
                        if not active:
                            continue
                        l = stream
                        for gi in range(3):
                            cc = gi * 512 + stream * 256
                            bias = (bias_val if uniform_bias else
                                    bsb[:, (di * 2 + l) * 4 + gi:
                                        (di * 2 + l) * 4 + gi + 1])
                            nc.scalar.activation(
                                gates[:, cc:cc + 256],
                                sig_ps[:, cc:cc + 256], AF.Sigmoid, bias=bias)
                        cc = 1536 + stream * 256
                        bias = (bias_val if uniform_bias else
                                bsb[:, (di * 2 + l) * 4 + 3:
                                    (di * 2 + l) * 4 + 4])
                        nc.scalar.activation(
                            gates[:, cc:cc + 256],
                            g_ps[:, stream * 256:stream * 256 + 256],
                            AF.Tanh, bias=bias)

                # ---- DVE combine (pair-merged) ----
                sc = nc.named_scope("combine"); sc.__enter__()
                c_t = c_pair[di]
                tt_t = tmp.tile([128, 2 * BCOL], DT2, tag="tt")
                nc.vector.tensor_mul(tt_t[:, lo:hi],
                                     gates[:, 512 + lo:512 + hi],
                                     c_t[:, lo:hi])
                p_t = tmp.tile([128, 2 * BCOL], DT2, tag="p")
                nc.vector.tensor_mul(p_t[:, lo:hi], gates[:, lo:hi],
                                     gates[:, 1536 + lo:1536 + hi])
                nc.vector.tensor_add(c_t[:, lo:hi], tt_t[:, lo:hi],
                                     p_t[:, lo:hi])
                sc.__exit__(None, None, None)

                # o-gate sigmoid: consumed only by next block's h-mul; fills
                # the ACT gap while DVE finishes the c update
                if sig_o is not None:
                    with nc.named_scope("sigo"):
                        nc.scalar.activation(sig_o[0][:, 1024:1536],
                                             sig_o[1][:, 1024:1536],
                                             AF.Sigmoid, bias=bias_val)

                # ---- late tail of superstep s-1: merge + store ----
                if merge is not None:
                    finish_tail(merge)

                # ---- x pipeline for superstep s+2 ----
                if s + 2 < NS:
                    x_prep(s + 2)

                pending = (di, st0, st1, tt1, gates, lo, hi)

                # ---- gate matmuls for superstep s+1 ----
                emit_mms(s + 1)

              if pending is not None:
                merge = finish_head(pending)
                if merge is not None:
                    finish_tail(merge)
                pending = None
            if loop_R:
                nc.sync.dma_start(dummyout.ap(), cnt[:])
    nc.compile()
    return nc


def _prep_weights(Wf, Uf, Wb, Ub):
    wts = np.zeros((2, 2, 8, 128, 128), dtype=DT2_NP)
    for di, (Wd, Ud) in enumerate(((Wf, Uf), (Wb, Ub))):
        for l in range(2):
            for si, sname in enumerate(SLOTS):
                mat = Wd[l] if sname[0] == "W" else Ud[l]
                g = GATE_COLS[sname[1]]
                wts[di, l, si] = np.asarray(
                    mat[:, g:g + 128], dtype=np.float32).astype(DT2_NP)
    return wts


def _prep_aux(bf, bb):
    eye = np.eye(128, dtype=np.float32)
    ident2 = np.concatenate([eye, 0.5 * eye], axis=1).astype(DT2_NP)
    biases = np.zeros((128, 16), dtype=np.float32)
    for di, bd in enumerate((bf, bb)):
        for l in range(2):
            for gi, gname in enumerate(("i", "f", "o", "g")):
                g = GATE_COLS[gname]
                biases[:, (di * 2 + l) * 4 + gi] = bd[l, g:g + 128]
    return ident2, biases


def kernel(x, Wf, Uf, bf, Wb, Ub, bb):
    x = np.ascontiguousarray(np.asarray(x, dtype=np.float32))
    bf = np.asarray(bf, dtype=np.float32)
    bb = np.asarray(bb, dtype=np.float32)

    bval = float(bf.flat[0])
    uniform = bool(np.all(bf == bval) and np.all(bb == bval))

    key = (uniform, bval if uniform else None)
    if key not in _cache:
        _cache[key] = _build(uniform, bval if uniform else 0.0)
    nc = _cache[key]

    wts = _prep_weights(Wf, Uf, Wb, Ub)
    ident2, biases = _prep_aux(bf, bb)

    in_maps = []
    for c in range(NCORES):
        in_maps.append({
            "xs": x[c * BL:(c + 1) * BL],
            "wts": wts,
            "ident2": ident2,
            "biases": biases,
        })
    res = run_bass_kernel_spmd(nc, in_maps, core_ids=list(range(NCORES)))
    return np.concatenate([res.results[c]["out"] for c in range(NCORES)],
                          axis=0).astype(np.float32)



# revision 3
# speedup vs baseline: 1.0420x; 1.0420x over previous
"""Trainium2 Bass kernel for nn_BiLSTM_57440892617018.

2-layer bidirectional LSTM (independent fw / bw stacks, merge_mode='ave'),
B=2048, T=200, D=U=128. Data-parallel over batch across 8 NeuronCores.

Per-core structure: ONE software-pipelined loop, 2 recurrent streams per
superstep (even = fw pair L0f/L1f with L1 lagging one timestep, odd = bw
pair on the time-reversed sequence). The schedule is built around keeping
the ACT engine (the busy-time bottleneck: 5 activations x 256 batch cols
per stream-timestep at ~0.83ns/col) 100% busy:

  ACT order per superstep s: [tanh_c(s-1) | sig_if(s) | tanh_g(s) | sig_o(s)]

tanh_c of the previous superstep runs while THIS superstep's DVE combine
(c = f*c + i*g) finishes, so ACT never waits on DVE. All DVE elementwise
work is emitted as scalar_tensor_tensor (4x DVE mode for fp16 SBUF
operands). x is pre-transposed on the host to [D, T, B] fp16 so DMA loads
land directly as matmul-ready [d, b] tiles (no PE transposes, no DVE
copies). PE runs fp16 matmuls: per superstep 16 gate matmuls (x/W first,
then h/U, g and o last for PSUM WAR friendliness) plus the layer-1 output
merge (0.5*I matmuls, which also produce the [b, d] layout the output DMA
needs). The PSUM->SBUF output copy runs on the otherwise-idle GPSIMD
engine.
"""

import numpy as np
import ml_dtypes

import concourse.bass as bass
import concourse.tile as tile
from concourse import bacc, mybir
from concourse.bass_utils import run_bass_kernel_spmd

F32 = mybir.dt.float32
DT2 = mybir.dt.float16
DT2_NP = np.float16
AF = mybir.ActivationFunctionType
ALU = mybir.AluOpType

B, T, D, U = 2048, 200, 128, 128
NCORES = 8
BL = B // NCORES          # 256 batch per core
NB = BL // 128            # 2 b-tiles
BCOL = 128 * NB           # 256 free columns (batch)

# gate order inside the reference 4u axis: i, f, g, o
GATE_COLS = {"i": 0, "f": 128, "g": 256, "o": 384}
SLOTS = ["Wi", "Wf", "Wo", "Wg", "Ui", "Uf", "Uo", "Ug"]
# W slot per gate / U slot per gate (indices into SLOTS)
WSLOT = {"i": 0, "f": 1, "g": 3, "o": 2}
USLOT = {"i": 4, "f": 5, "g": 7, "o": 6}

_cache = {}


def _wcol(di, l, slot):
    return ((di * 2 + l) * 8 + slot) * 128


def _build(uniform_bias, bias_val, Tn=T, reps=1, loop_R=0, tiny_x=False):
    nc = bacc.Bacc("TRN2", target_bir_lowering=False, debug=False,
                   num_devices=NCORES)
    Th = Tn // 2

    # x pre-transposed on host: [D, T, B] fp16 (contiguous b per (d, t))
    xs = nc.dram_tensor("xs", [D, 1 if tiny_x else Tn, BCOL], DT2,
                        kind="ExternalInput")
    wts = nc.dram_tensor("wts", [2, 2, 8, 128, 128], DT2, kind="ExternalInput")
    halfeye = nc.dram_tensor("halfeye", [128, 128], DT2, kind="ExternalInput")
    biases = nc.dram_tensor("biases", [128, 16], F32, kind="ExternalInput")
    if loop_R:
        dummyout = nc.dram_tensor("dummyout", [128, 4], F32,
                                  kind="ExternalOutput")
    else:
        out = nc.dram_tensor("out", [BL, Tn, D], F32, kind="ExternalOutput")

    NS = 2 * Tn + 2   # supersteps

    with tile.TileContext(nc) as tc:
        with (
            tc.tile_pool(name="wpool", bufs=1) as wpool,
            tc.tile_pool(name="stage", bufs=1) as stage,
            tc.tile_pool(name="xpool", bufs=6) as xpool,
            tc.tile_pool(name="hpool", bufs=4) as hpool,
            tc.tile_pool(name="cpool", bufs=1) as cpool,
            tc.tile_pool(name="gsb", bufs=3) as gsb,
            tc.tile_pool(name="tcp", bufs=2) as tcp,
            tc.tile_pool(name="ttp", bufs=2) as ttp,
            tc.tile_pool(name="ppl", bufs=2) as ppl,
            tc.tile_pool(name="outp", bufs=4) as outp,
            tc.tile_pool(name="dramp", bufs=1, space="DRAM") as dramp,
            tc.tile_pool(name="pif", bufs=2, space="PSUM") as pif,
            tc.tile_pool(name="pg", bufs=1, space="PSUM") as pg,
            tc.tile_pool(name="po", bufs=1, space="PSUM") as po,
            tc.tile_pool(name="pmg", bufs=2, space="PSUM") as pmg,
        ):
            # ---- constants / weights ----
            wslab = wpool.tile([128, 4096], DT2)
            for di in range(2):
                for l in range(2):
                    base = _wcol(di, l, 0)
                    nc.sync.dma_start(
                        wslab[:, base:base + 1024].rearrange(
                            "k (s m) -> k s m", s=8),
                        wts.ap()[di, l].rearrange("s k m -> k s m"))
            heye = wpool.tile([128, 128], DT2)
            nc.sync.dma_start(heye[:], halfeye.ap())
            bsb = wpool.tile([128, 16], F32)
            nc.sync.dma_start(bsb[:], biases.ap())

            if loop_R:
                out_int = dramp.tile([BL, Tn, D], F32)

                def out_ap():
                    return out_int[:]
            else:
                def out_ap():
                    return out.ap()

            z0 = wpool.tile([128, BCOL], DT2)
            nc.gpsimd.memset(z0[:], 0.0)

            # persistent cell states per direction: [c0|c1] pair tiles
            c_pair = {}
            for di in range(2):
                ct = cpool.tile([128, 2 * BCOL], DT2, tag=f"c{di}")
                nc.gpsimd.memset(ct[:], 0.0)
                c_pair[di] = ct

            # SBUF staging for layer-1 outputs awaiting merge
            stage_f = stage.tile([128, Th * BCOL], DT2, tag="stf")
            stage_b = stage.tile([128, Th * BCOL], DT2, tag="stb")

            h_prev = {(0, 0): z0[:], (0, 1): z0[:],
                      (1, 0): z0[:], (1, 1): z0[:]}
            xT_ready = {}
            mm_ctx = {}             # s -> (pif_t, pg_t, po_t)

            def x_load(s):
                m = s // 2
                if m >= Tn:
                    return
                di = s % 2
                tsrc = 0 if tiny_x else (m if di == 0 else Tn - 1 - m)
                with nc.named_scope("xload"):
                    xt = xpool.tile([128, BCOL], DT2)
                    nc.sync.dma_start(xt[:], xs.ap()[:, tsrc, :])
                xT_ready[s] = xt

            def stt(out_ap_, in0, in1, op1):
                nc.vector.scalar_tensor_tensor(
                    out_ap_, in0, 1.0, in1, ALU.mult, op1)

            def emit_mms(s):
                """Gate matmuls for superstep s. PE order: x-dependent W
                matmuls first (PE idle time), then h-dependent U matmuls,
                then g and o last (their PSUM tiles are single-buffered and
                WAR-wait on this superstep's ACT reads)."""
                di = s % 2
                m = s // 2
                st0 = m < Tn
                st1 = 1 <= m <= Tn
                if not (st0 or st1) or s >= NS:
                    return
                sc = nc.named_scope("gatemm"); sc.__enter__()
                pif_t = pif.tile([128, 1024], F32)
                pg_t = pg.tile([128, 512], F32)
                po_t = po.tile([128, 512], F32)
                rhs = {}
                if st0:
                    rhs[0] = (xT_ready.pop(s)[:], h_prev[(di, 0)])
                if st1:
                    rhs[1] = (h_prev[(di, 0)], h_prev[(di, 1)])

                def mm(dst, stream, slot, r, start, stop):
                    nc.tensor.matmul(
                        dst, wslab[:, _wcol(di, stream, slot):
                                   _wcol(di, stream, slot) + 128],
                        r, start=start, stop=stop)

                # i, f -> pif [i0|i1|f0|f1]
                for stream in rhs:
                    for gn, base in (("i", 0), ("f", 512)):
                        dst = pif_t[:, base + stream * 256:
                                    base + stream * 256 + 256]
                        mm(dst, stream, WSLOT[gn], rhs[stream][0], True, False)
                    for gn, base in (("i", 0), ("f", 512)):
                        dst = pif_t[:, base + stream * 256:
                                    base + stream * 256 + 256]
                        mm(dst, stream, USLOT[gn], rhs[stream][1], False, True)
                # g -> pg [g0|g1]
                for stream in rhs:
                    dst = pg_t[:, stream * 256:stream * 256 + 256]
                    mm(dst, stream, WSLOT["g"], rhs[stream][0], True, False)
                    mm(dst, stream, USLOT["g"], rhs[stream][1], False, True)
                # o -> po [o0|o1]
                for stream in rhs:
                    dst = po_t[:, stream * 256:stream * 256 + 256]
                    mm(dst, stream, WSLOT["o"], rhs[stream][0], True, False)
                    mm(dst, stream, USLOT["o"], rhs[stream][1], False, True)
                sc.__exit__(None, None, None)
                mm_ctx[s] = (pif_t, pg_t, po_t)

            def finish_prev(pend):
                """tanh(c) + h-muls of superstep s-1 (ACT slot 1 + DVE)."""
                (di_p, st0, st1, tt1, gates_p, lo, hi) = pend
                c_t = c_pair[di_p]
                tc_t = tcp.tile([128, 2 * BCOL], DT2)
                with nc.named_scope("tanhc"):
                    nc.scalar.activation(tc_t[:, lo:hi], c_t[:, lo:hi],
                                         AF.Tanh)
                merge = None
                with nc.named_scope("hmul"):
                    if st0:
                        h_t = hpool.tile([128, BCOL], DT2, tag="h0")
                        stt(h_t[:], gates_p[:, 1024:1280], tc_t[:, 0:BCOL],
                            ALU.mult)
                        h_prev[(di_p, 0)] = h_t[:]
                    if st1:
                        o_ap = gates_p[:, 1280:1536]
                        stg = stage_f if di_p == 0 else stage_b
                        stage_this = (tt1 < Th) if di_p == 0 else (tt1 >= Th)
                        if stage_this:
                            soff = (tt1 if di_p == 0 else tt1 - Th) * BCOL
                            dst = stg[:, soff:soff + BCOL]
                            stt(dst, o_ap, tc_t[:, BCOL:2 * BCOL], ALU.mult)
                            h_prev[(di_p, 1)] = dst
                        else:
                            h_t = hpool.tile([128, BCOL], DT2, tag="h1")
                            stt(h_t[:], o_ap, tc_t[:, BCOL:2 * BCOL],
                                ALU.mult)
                            h_prev[(di_p, 1)] = h_t[:]
                            merge = (di_p, tt1, h_t)
                return merge

            def emit_acts(s, st0, st1, gates, lo, hi):
                """sig_if, tanh_g, sig_o for superstep s (ACT slots 2-4)."""
                pif_t, pg_t, po_t = mm_ctx.pop(s)
                di = s % 2
                if uniform_bias and st0 and st1:
                    with nc.named_scope("sigif"):
                        nc.scalar.activation(gates[:, 0:1024], pif_t[:],
                                             AF.Sigmoid, bias=bias_val)
                    with nc.named_scope("tanhg"):
                        nc.scalar.activation(gates[:, 1536:2048], pg_t[:],
                                             AF.Tanh, bias=bias_val)
                    with nc.named_scope("sigo"):
                        nc.scalar.activation(gates[:, 1024:1536], po_t[:],
                                             AF.Sigmoid, bias=bias_val)
                else:
                    for stream, active in ((0, st0), (1, st1)):
                        if not active:
                            continue
                        l = stream

                        def bias_for(gi):
                            return (bias_val if uniform_bias else
                                    bsb[:, (di * 2 + l) * 4 + gi:
                                        (di * 2 + l) * 4 + gi + 1])

                        cc = stream * 256
                        nc.scalar.activation(gates[:, cc:cc + 256],
                                             pif_t[:, cc:cc + 256],
                                             AF.Sigmoid, bias=bias_for(0))
                        nc.scalar.activation(gates[:, 512 + cc:512 + cc + 256],
                                             pif_t[:, 512 + cc:512 + cc + 256],
                                             AF.Sigmoid, bias=bias_for(1))
                        nc.scalar.activation(gates[:, 1536 + cc:1536 + cc + 256],
                                             pg_t[:, cc:cc + 256],
                                             AF.Tanh, bias=bias_for(3))
                        nc.scalar.activation(gates[:, 1024 + cc:1024 + cc + 256],
                                             po_t[:, cc:cc + 256],
                                             AF.Sigmoid, bias=bias_for(2))

            def emit_combine(s, gates, lo, hi):
                """DVE: c = f*c + i*g (three 4x STT ops)."""
                sc = nc.named_scope("combine"); sc.__enter__()
                di = s % 2
                c_t = c_pair[di]
                tt_t = ttp.tile([128, 2 * BCOL], DT2)
                p_t = ppl.tile([128, 2 * BCOL], DT2)
                stt(tt_t[:, lo:hi], gates[:, 512 + lo:512 + hi],
                    c_t[:, lo:hi], ALU.mult)
                stt(p_t[:, lo:hi], gates[:, lo:hi],
                    gates[:, 1536 + lo:1536 + hi], ALU.mult)
                stt(c_t[:, lo:hi], tt_t[:, lo:hi], p_t[:, lo:hi], ALU.add)
                sc.__exit__(None, None, None)

            def finish_merge(merge):
                """Layer-1 output merge on PE + GPSIMD copy + store."""
                di_p, tt1, h_t = merge
                sc = nc.named_scope("mergeout"); sc.__enter__()
                ostg = stage_b if di_p == 0 else stage_f
                ooff = (tt1 - Th if di_p == 0 else tt1) * BCOL
                other = ostg[:, ooff:ooff + BCOL]
                pm = pmg.tile([128, BCOL], F32)
                for j in range(NB):
                    mdst = pm[:, j * 128:j * 128 + 128]
                    nc.tensor.matmul(
                        mdst, h_t[:, j * 128:j * 128 + 128],
                        heye[:], start=True, stop=False)
                    nc.tensor.matmul(
                        mdst, other[:, j * 128:j * 128 + 128],
                        heye[:], start=False, stop=True)
                ost = outp.tile([128, BCOL], F32)
                nc.vector.tensor_copy(ost[:], pm[:])
                nc.sync.dma_start(
                    out_ap()[:, tt1, :].rearrange("(j p) d -> p j d", j=NB),
                    ost[:].rearrange("p (j d) -> p j d", j=NB))
                sc.__exit__(None, None, None)

            import contextlib
            pending = None          # state of superstep s-1
            if loop_R:
                cnt = wpool.tile([128, 4], F32, tag="cnt")
                nc.gpsimd.memset(cnt[:], 0.0)
            loop_cm = tc.For_i(0, loop_R, 1) if loop_R else \
                contextlib.nullcontext()
            with loop_cm:
             if loop_R:
                 nc.vector.tensor_scalar_add(cnt[:], cnt[:], 1.0)
             for rep in range(reps):
              if rep > 0 or loop_R:
                for di_ in range(2):
                    nc.gpsimd.memset(c_pair[di_][:], 0.0)
                h_prev.update({(0, 0): z0[:], (0, 1): z0[:],
                               (1, 0): z0[:], (1, 1): z0[:]})
                pending = None
              for s in range(NS):
                di = s % 2
                m = s // 2
                st0 = m < Tn
                st1 = 1 <= m <= Tn
                t1 = m - 1
                tt1 = t1 if di == 0 else Tn - 1 - t1
                lo = 0 if st0 else BCOL
                hi = 2 * BCOL if st1 else BCOL

                # ---- prologue / x prefetch ----
                if s == 0:
                    for ps in range(4):
                        x_load(ps)
                    emit_mms(0)
                if s + 4 < NS:
                    x_load(s + 4)

                # ---- ACT slot 1 + DVE h-muls: finish superstep s-1 ----
                merge = None
                if pending is not None:
                    merge = finish_prev(pending)
                    pending = None

                # ---- ACT slots 2-4: activations for superstep s ----
                gates = gsb.tile([128, 2048], DT2)
                emit_acts(s, st0, st1, gates, lo, hi)

                # ---- DVE: c update ----
                emit_combine(s, gates, lo, hi)

                # ---- PE: gate matmuls for superstep s+1 ----
                emit_mms(s + 1)

                # ---- PE merge + GPSIMD copy + store of superstep s-1 ----
                if merge is not None:
                    finish_merge(merge)

                pending = (di, st0, st1, tt1, gates, lo, hi)

              if pending is not None:
                merge = finish_prev(pending)
                if merge is not None:
                    finish_merge(merge)
                pending = None
             if loop_R:
                nc.sync.dma_start(dummyout.ap(), cnt[:])
    nc.compile()
    return nc


def _prep_weights(Wf, Uf, Wb, Ub):
    wts = np.zeros((2, 2, 8, 128, 128), dtype=DT2_NP)
    for di, (Wd, Ud) in enumerate(((Wf, Uf), (Wb, Ub))):
        for l in range(2):
            for si, sname in enumerate(SLOTS):
                mat = Wd[l] if sname[0] == "W" else Ud[l]
                g = GATE_COLS[sname[1]]
                wts[di, l, si] = np.asarray(
                    mat[:, g:g + 128], dtype=np.float32).astype(DT2_NP)
    return wts


def _prep_aux(bf, bb):
    halfeye = (0.5 * np.eye(128, dtype=np.float32)).astype(DT2_NP)
    biases = np.zeros((128, 16), dtype=np.float32)
    for di, bd in enumerate((bf, bb)):
        for l in range(2):
            for gi, gname in enumerate(("i", "f", "o", "g")):
                g = GATE_COLS[gname]
                biases[:, (di * 2 + l) * 4 + gi] = bd[l, g:g + 128]
    return halfeye, biases


def _prep_x(x):
    """Per-core [D, T, BL] fp16 transposes of the batch shards."""
    x16 = np.asarray(x, dtype=np.float32).astype(DT2_NP)
    return [np.ascontiguousarray(
        x16[c * BL:(c + 1) * BL].transpose(2, 1, 0))
        for c in range(NCORES)]


def kernel(x, Wf, Uf, bf, Wb, Ub, bb):
    bf = np.asarray(bf, dtype=np.float32)
    bb = np.asarray(bb, dtype=np.float32)

    bval = float(bf.flat[0])
    uniform = bool(np.all(bf == bval) and np.all(bb == bval))

    key = (uniform, bval if uniform else None)
    if key not in _cache:
        _cache[key] = _build(uniform, bval if uniform else 0.0)
    nc = _cache[key]

    wts = _prep_weights(Wf, Uf, Wb, Ub)
    halfeye, biases = _prep_aux(bf, bb)
    xcores = _prep_x(x)

    in_maps = []
    for c in range(NCORES):
        in_maps.append({
            "xs": xcores[c],
            "wts": wts,
            "halfeye": halfeye,
            "biases": biases,
        })
    res = run_bass_kernel_spmd(nc, in_maps, core_ids=list(range(NCORES)))
    return np.concatenate([res.results[c]["out"] for c in range(NCORES)],
                          axis=0).astype(np.float32)


# revision 6
# speedup vs baseline: 1.1346x; 1.0889x over previous
"""Trainium2 Bass kernel for nn_BiLSTM_57440892617018.

2-layer bidirectional LSTM (independent fw / bw stacks, merge_mode='ave'),
B=2048, T=200, D=U=128. Data-parallel over batch across 8 NeuronCores.

Per-core structure: ONE software-pipelined loop, 2 recurrent streams per
superstep (even = fw pair L0f/L1f with L1 lagging one timestep, odd = bw
pair on the time-reversed sequence). The schedule is built around keeping
the ACT engine (the busy-time bottleneck: 5 activations x 256 batch cols
per stream-timestep at ~0.83ns/col) 100% busy:

  ACT order per superstep s: [tanh_c(s-1) | sig_if(s) | tanh_g(s) | sig_o(s)]

tanh_c of the previous superstep runs while THIS superstep's DVE combine
(c = f*c + i*g) finishes, so ACT never waits on DVE. All DVE elementwise
work is emitted as scalar_tensor_tensor (4x DVE mode for fp16 SBUF
operands). x is pre-transposed on the host to [D, T, B] fp16 so DMA loads
land directly as matmul-ready [d, b] tiles (no PE transposes, no DVE
copies). PE runs fp16 matmuls: per superstep 16 gate matmuls (x/W first,
then h/U, g and o last for PSUM WAR friendliness) plus the layer-1 output
merge (0.5*I matmuls, which also produce the [b, d] layout the output DMA
needs). The PSUM->SBUF output copy runs on the otherwise-idle GPSIMD
engine.
"""

import numpy as np
import ml_dtypes

import concourse.bass as bass
import concourse.tile as tile
from concourse import bacc, mybir
from concourse.bass_utils import run_bass_kernel_spmd

F32 = mybir.dt.float32
DT2 = mybir.dt.float16
DT2_NP = np.float16
AF = mybir.ActivationFunctionType
ALU = mybir.AluOpType

B, T, D, U = 2048, 200, 128, 128
NCORES = 8
BL = B // NCORES          # 256 batch per core
NB = BL // 128            # 2 b-tiles
BCOL = 128 * NB           # 256 free columns (batch)

# gate order inside the reference 4u axis: i, f, g, o
GATE_COLS = {"i": 0, "f": 128, "g": 256, "o": 384}
SLOTS = ["Wi", "Wf", "Wo", "Wg", "Ui", "Uf", "Uo", "Ug"]
# W slot per gate / U slot per gate (indices into SLOTS)
WSLOT = {"i": 0, "f": 1, "g": 3, "o": 2}
USLOT = {"i": 4, "f": 5, "g": 7, "o": 6}

_cache = {}


def _wcol(di, l, slot):
    return ((di * 2 + l) * 8 + slot) * 128


def _build(uniform_bias, bias_val, Tn=T, reps=1, loop_R=0, tiny_x=False):
    nc = bacc.Bacc("TRN2", target_bir_lowering=False, debug=False,
                   num_devices=NCORES)
    Th = Tn // 2

    # x pre-transposed on host: [D, T, B] fp16 (contiguous b per (d, t))
    xs = nc.dram_tensor("xs", [D, 1 if tiny_x else Tn, BCOL], DT2,
                        kind="ExternalInput")
    wts = nc.dram_tensor("wts", [2, 2, 8, 128, 128], DT2, kind="ExternalInput")
    halfeye = nc.dram_tensor("halfeye", [128, 128], DT2, kind="ExternalInput")
    biases = nc.dram_tensor("biases", [128, 16], F32, kind="ExternalInput")
    if loop_R:
        dummyout = nc.dram_tensor("dummyout", [128, 4], F32,
                                  kind="ExternalOutput")
    else:
        out = nc.dram_tensor("out", [BL, Tn, D], F32, kind="ExternalOutput")

    NS = 2 * Tn + 2   # supersteps

    with tile.TileContext(nc) as tc:
        with (
            tc.tile_pool(name="wpool", bufs=1) as wpool,
            tc.tile_pool(name="stage", bufs=1) as stage,
            tc.tile_pool(name="xpool", bufs=6) as xpool,
            tc.tile_pool(name="hpool", bufs=4) as hpool,
            tc.tile_pool(name="cpool", bufs=1) as cpool,
            tc.tile_pool(name="gsb", bufs=3) as gsb,
            tc.tile_pool(name="tcp", bufs=2) as tcp,
            tc.tile_pool(name="ttp", bufs=2) as ttp,
            tc.tile_pool(name="ppl", bufs=2) as ppl,
            tc.tile_pool(name="outp", bufs=4) as outp,
            tc.tile_pool(name="dramp", bufs=1, space="DRAM") as dramp,
            tc.tile_pool(name="pif", bufs=2, space="PSUM") as pif,
            tc.tile_pool(name="pg", bufs=1, space="PSUM") as pg,
            tc.tile_pool(name="po", bufs=1, space="PSUM") as po,
            tc.tile_pool(name="pmg", bufs=2, space="PSUM") as pmg,
        ):
            # ---- constants / weights ----
            wslab = wpool.tile([128, 4096], DT2)
            for di in range(2):
                for l in range(2):
                    base = _wcol(di, l, 0)
                    nc.sync.dma_start(
                        wslab[:, base:base + 1024].rearrange(
                            "k (s m) -> k s m", s=8),
                        wts.ap()[di, l].rearrange("s k m -> k s m"))
            heye = wpool.tile([128, 128], DT2)
            nc.sync.dma_start(heye[:], halfeye.ap())
            bsb = wpool.tile([128, 16], F32)
            nc.sync.dma_start(bsb[:], biases.ap())

            if loop_R:
                out_int = dramp.tile([BL, Tn, D], F32)

                def out_ap():
                    return out_int[:]
            else:
                def out_ap():
                    return out.ap()

            z0 = wpool.tile([128, BCOL], DT2)
            nc.gpsimd.memset(z0[:], 0.0)

            # persistent cell states per direction: [c0|c1] pair tiles
            c_pair = {}
            for di in range(2):
                ct = cpool.tile([128, 2 * BCOL], DT2, tag=f"c{di}")
                nc.gpsimd.memset(ct[:], 0.0)
                c_pair[di] = ct

            # SBUF staging for layer-1 outputs awaiting merge
            stage_f = stage.tile([128, Th * BCOL], DT2, tag="stf")
            stage_b = stage.tile([128, Th * BCOL], DT2, tag="stb")

            h_prev = {(0, 0): z0[:], (0, 1): z0[:],
                      (1, 0): z0[:], (1, 1): z0[:]}
            xT_ready = {}
            mm_ctx = {}             # s -> (pif_t, pg_t, po_t)

            def x_load(s):
                m = s // 2
                if m >= Tn:
                    return
                di = s % 2
                tsrc = 0 if tiny_x else (m if di == 0 else Tn - 1 - m)
                with nc.named_scope("xload"):
                    xt = xpool.tile([128, BCOL], DT2)
                    nc.sync.dma_start(xt[:], xs.ap()[:, tsrc, :])
                xT_ready[s] = xt

            def stt(out_ap_, in0, in1, op1):
                # tensor_tensor runs in the 2x DVE mode for fp16 operands;
                # the 3-tensor scalar_tensor_tensor form would run at 1x.
                if op1 == ALU.add:
                    nc.vector.tensor_add(out_ap_, in0, in1)
                else:
                    nc.vector.tensor_mul(out_ap_, in0, in1)

            def emit_mms(s):
                """Gate matmuls for superstep s. PE order: x-dependent W
                matmuls first (PE idle time), then h-dependent U matmuls,
                then g and o last (their PSUM tiles are single-buffered and
                WAR-wait on this superstep's ACT reads)."""
                di = s % 2
                m = s // 2
                st0 = m < Tn
                st1 = 1 <= m <= Tn
                if not (st0 or st1) or s >= NS:
                    return
                sc = nc.named_scope("gatemm"); sc.__enter__()
                pif_t = pif.tile([128, 1024], F32)
                pg_t = pg.tile([128, 512], F32)
                po_t = po.tile([128, 512], F32)
                rhs = {}
                if st0:
                    rhs[0] = (xT_ready.pop(s)[:], h_prev[(di, 0)])
                if st1:
                    rhs[1] = (h_prev[(di, 0)], h_prev[(di, 1)])

                def mm(dst, stream, slot, r, start, stop):
                    nc.tensor.matmul(
                        dst, wslab[:, _wcol(di, stream, slot):
                                   _wcol(di, stream, slot) + 128],
                        r, start=start, stop=stop)

                # i, f -> pif [i0|i1|f0|f1]
                for stream in rhs:
                    for gn, base in (("i", 0), ("f", 512)):
                        dst = pif_t[:, base + stream * 256:
                                    base + stream * 256 + 256]
                        mm(dst, stream, WSLOT[gn], rhs[stream][0], True, False)
                    for gn, base in (("i", 0), ("f", 512)):
                        dst = pif_t[:, base + stream * 256:
                                    base + stream * 256 + 256]
                        mm(dst, stream, USLOT[gn], rhs[stream][1], False, True)
                # g -> pg [g0|g1]
                for stream in rhs:
                    dst = pg_t[:, stream * 256:stream * 256 + 256]
                    mm(dst, stream, WSLOT["g"], rhs[stream][0], True, False)
                    mm(dst, stream, USLOT["g"], rhs[stream][1], False, True)
                # o -> po [o0|o1]
                for stream in rhs:
                    dst = po_t[:, stream * 256:stream * 256 + 256]
                    mm(dst, stream, WSLOT["o"], rhs[stream][0], True, False)
                    mm(dst, stream, USLOT["o"], rhs[stream][1], False, True)
                sc.__exit__(None, None, None)
                mm_ctx[s] = (pif_t, pg_t, po_t)

            def finish_prev(pend):
                """tanh(c) + h-muls of superstep s-1 (ACT slot 1 + DVE)."""
                (di_p, st0, st1, tt1, gates_p, lo, hi) = pend
                c_t = c_pair[di_p]
                tc_t = tcp.tile([128, 2 * BCOL], DT2)
                with nc.named_scope("tanhc"):
                    nc.scalar.activation(tc_t[:, lo:hi], c_t[:, lo:hi],
                                         AF.Tanh)
                merge = None
                with nc.named_scope("hmul"):
                    if st0:
                        h_t = hpool.tile([128, BCOL], DT2, tag="h0")
                        stt(h_t[:], gates_p[:, 1024:1280], tc_t[:, 0:BCOL],
                            ALU.mult)
                        h_prev[(di_p, 0)] = h_t[:]
                    if st1:
                        o_ap = gates_p[:, 1280:1536]
                        stg = stage_f if di_p == 0 else stage_b
                        stage_this = (tt1 < Th) if di_p == 0 else (tt1 >= Th)
                        if stage_this:
                            soff = (tt1 if di_p == 0 else tt1 - Th) * BCOL
                            dst = stg[:, soff:soff + BCOL]
                            stt(dst, o_ap, tc_t[:, BCOL:2 * BCOL], ALU.mult)
                            h_prev[(di_p, 1)] = dst
                        else:
                            h_t = hpool.tile([128, BCOL], DT2, tag="h1")
                            stt(h_t[:], o_ap, tc_t[:, BCOL:2 * BCOL],
                                ALU.mult)
                            h_prev[(di_p, 1)] = h_t[:]
                            merge = (di_p, tt1, h_t)
                return merge

            def bias_for(di, l, gi):
                return (bias_val if uniform_bias else
                        bsb[:, (di * 2 + l) * 4 + gi:
                            (di * 2 + l) * 4 + gi + 1])

            def emit_sig_if(s, st0, st1, gates, pif_t):
                """ACT slot 1: sigmoid over [i0|i1|f0|f1]."""
                di = s % 2
                if uniform_bias and st0 and st1:
                    with nc.named_scope("sigif"):
                        nc.scalar.activation(gates[:, 0:1024], pif_t[:],
                                             AF.Sigmoid, bias=bias_val)
                else:
                    for stream, active in ((0, st0), (1, st1)):
                        if not active:
                            continue
                        cc = stream * 256
                        nc.scalar.activation(gates[:, cc:cc + 256],
                                             pif_t[:, cc:cc + 256],
                                             AF.Sigmoid,
                                             bias=bias_for(di, stream, 0))
                        nc.scalar.activation(gates[:, 512 + cc:512 + cc + 256],
                                             pif_t[:, 512 + cc:512 + cc + 256],
                                             AF.Sigmoid,
                                             bias=bias_for(di, stream, 1))

            def emit_tanhg(s, st0, st1, gates, pg_t):
                """ACT slot 3: tanh over [g0|g1]."""
                di = s % 2
                if uniform_bias and st0 and st1:
                    with nc.named_scope("tanhg"):
                        nc.scalar.activation(gates[:, 1536:2048], pg_t[:],
                                             AF.Tanh, bias=bias_val)
                else:
                    for stream, active in ((0, st0), (1, st1)):
                        if not active:
                            continue
                        cc = stream * 256
                        nc.scalar.activation(gates[:, 1536 + cc:1536 + cc + 256],
                                             pg_t[:, cc:cc + 256],
                                             AF.Tanh,
                                             bias=bias_for(di, stream, 3))

            def emit_sigo(s, st0, st1, gates, po_t):
                """ACT slot 4: sigmoid over [o0|o1]."""
                di = s % 2
                if uniform_bias and st0 and st1:
                    with nc.named_scope("sigo"):
                        nc.scalar.activation(gates[:, 1024:1536], po_t[:],
                                             AF.Sigmoid, bias=bias_val)
                else:
                    for stream, active in ((0, st0), (1, st1)):
                        if not active:
                            continue
                        cc = stream * 256
                        nc.scalar.activation(gates[:, 1024 + cc:1024 + cc + 256],
                                             po_t[:, cc:cc + 256],
                                             AF.Sigmoid,
                                             bias=bias_for(di, stream, 2))

            def finish_merge(merge):
                """Layer-1 output merge on PE + GPSIMD copy + store."""
                di_p, tt1, h_t = merge
                sc = nc.named_scope("mergeout"); sc.__enter__()
                ostg = stage_b if di_p == 0 else stage_f
                ooff = (tt1 - Th if di_p == 0 else tt1) * BCOL
                other = ostg[:, ooff:ooff + BCOL]
                pm = pmg.tile([128, BCOL], F32)
                for j in range(NB):
                    mdst = pm[:, j * 128:j * 128 + 128]
                    nc.tensor.matmul(
                        mdst, h_t[:, j * 128:j * 128 + 128],
                        heye[:], start=True, stop=False)
                    nc.tensor.matmul(
                        mdst, other[:, j * 128:j * 128 + 128],
                        heye[:], start=False, stop=True)
                ost = outp.tile([128, BCOL], F32)
                nc.vector.tensor_copy(ost[:], pm[:])
                nc.sync.dma_start(
                    out_ap()[:, tt1, :].rearrange("(j p) d -> p j d", j=NB),
                    ost[:].rearrange("p (j d) -> p j d", j=NB))
                sc.__exit__(None, None, None)

            import contextlib
            pending = None          # state of superstep s-1
            if loop_R:
                cnt = wpool.tile([128, 4], F32, tag="cnt")
                nc.gpsimd.memset(cnt[:], 0.0)
            loop_cm = tc.For_i(0, loop_R, 1) if loop_R else \
                contextlib.nullcontext()
            with loop_cm:
             if loop_R:
                 nc.vector.tensor_scalar_add(cnt[:], cnt[:], 1.0)
             for rep in range(reps):
              if rep > 0 or loop_R:
                for di_ in range(2):
                    nc.gpsimd.memset(c_pair[di_][:], 0.0)
                h_prev.update({(0, 0): z0[:], (0, 1): z0[:],
                               (1, 0): z0[:], (1, 1): z0[:]})
                pending = None
              for s in range(NS):
                di = s % 2
                m = s // 2
                st0 = m < Tn
                st1 = 1 <= m <= Tn
                t1 = m - 1
                tt1 = t1 if di == 0 else Tn - 1 - t1
                lo = 0 if st0 else BCOL
                hi = 2 * BCOL if st1 else BCOL

                # ---- prologue / x prefetch ----
                if s == 0:
                    for ps in range(4):
                        x_load(ps)
                    emit_mms(0)
                if s + 4 < NS:
                    x_load(s + 4)

                pif_t, pg_t, po_t = mm_ctx.pop(s)
                gates = gsb.tile([128, 2048], DT2)
                c_t = c_pair[di]

                # ---- ACT slot 1: sig_if(s) ----
                emit_sig_if(s, st0, st1, gates, pif_t)

                # ---- DVE: tt(s) = f * c (before the h-muls in DVE order,
                # it only needs sig_if) ----
                tt_t = ttp.tile([128, 2 * BCOL], DT2)
                with nc.named_scope("ttmul"):
                    stt(tt_t[:, lo:hi], gates[:, 512 + lo:512 + hi],
                        c_t[:, lo:hi], ALU.mult)

                # ---- ACT slot 2 + DVE h-muls: finish superstep s-1 ----
                merge = None
                if pending is not None:
                    merge = finish_prev(pending)
                    pending = None

                # ---- ACT slot 3: tanhg(s) ----
                emit_tanhg(s, st0, st1, gates, pg_t)

                # ---- DVE: p(s) = i * g ; c(s) = tt + p ----
                sc = nc.named_scope("combine"); sc.__enter__()
                p_t = ppl.tile([128, 2 * BCOL], DT2)
                stt(p_t[:, lo:hi], gates[:, lo:hi],
                    gates[:, 1536 + lo:1536 + hi], ALU.mult)
                stt(c_t[:, lo:hi], tt_t[:, lo:hi], p_t[:, lo:hi], ALU.add)
                sc.__exit__(None, None, None)

                # ---- ACT slot 4: sigo(s) ----
                emit_sigo(s, st0, st1, gates, po_t)

                # ---- PE: gate matmuls for superstep s+1 ----
                emit_mms(s + 1)

                # ---- PE merge + DVE copy + store of superstep s-1 ----
                if merge is not None:
                    finish_merge(merge)

                pending = (di, st0, st1, tt1, gates, lo, hi)

              if pending is not None:
                merge = finish_prev(pending)
                if merge is not None:
                    finish_merge(merge)
                pending = None
             if loop_R:
                nc.sync.dma_start(dummyout.ap(), cnt[:])
    nc.compile()
    return nc


def _prep_weights(Wf, Uf, Wb, Ub):
    wts = np.zeros((2, 2, 8, 128, 128), dtype=DT2_NP)
    for di, (Wd, Ud) in enumerate(((Wf, Uf), (Wb, Ub))):
        for l in range(2):
            for si, sname in enumerate(SLOTS):
                mat = Wd[l] if sname[0] == "W" else Ud[l]
                g = GATE_COLS[sname[1]]
                wts[di, l, si] = np.asarray(
                    mat[:, g:g + 128], dtype=np.float32).astype(DT2_NP)
    return wts


def _prep_aux(bf, bb):
    halfeye = (0.5 * np.eye(128, dtype=np.float32)).astype(DT2_NP)
    biases = np.zeros((128, 16), dtype=np.float32)
    for di, bd in enumerate((bf, bb)):
        for l in range(2):
            for gi, gname in enumerate(("i", "f", "o", "g")):
                g = GATE_COLS[gname]
                biases[:, (di * 2 + l) * 4 + gi] = bd[l, g:g + 128]
    return halfeye, biases


def _prep_x(x):
    """Per-core [D, T, BL] fp16 transposes of the batch shards."""
    x16 = np.asarray(x, dtype=np.float32).astype(DT2_NP)
    return [np.ascontiguousarray(
        x16[c * BL:(c + 1) * BL].transpose(2, 1, 0))
        for c in range(NCORES)]


def kernel(x, Wf, Uf, bf, Wb, Ub, bb):
    bf = np.asarray(bf, dtype=np.float32)
    bb = np.asarray(bb, dtype=np.float32)

    bval = float(bf.flat[0])
    uniform = bool(np.all(bf == bval) and np.all(bb == bval))

    key = (uniform, bval if uniform else None)
    if key not in _cache:
        _cache[key] = _build(uniform, bval if uniform else 0.0)
    nc = _cache[key]

    wts = _prep_weights(Wf, Uf, Wb, Ub)
    halfeye, biases = _prep_aux(bf, bb)
    xcores = _prep_x(x)

    in_maps = []
    for c in range(NCORES):
        in_maps.append({
            "xs": xcores[c],
            "wts": wts,
            "halfeye": halfeye,
            "biases": biases,
        })
    res = run_bass_kernel_spmd(nc, in_maps, core_ids=list(range(NCORES)))
    return np.concatenate([res.results[c]["out"] for c in range(NCORES)],
                          axis=0).astype(np.float32)
